# revision 15
# baseline (speedup 1.0000x reference)
"""Expert-parallel MoE GLU MLP kernel for Trainium2.

8 experts -> 8 NeuronCores, one expert per core (no collectives needed).
Per core:  x (C,H) @ w_gate_up (H,2I) -> GLU -> (C,I) @ w_down (I,H) -> (C,H)

v2: all inputs are pre-cast to bf16 on the host (the kernel computed in
bf16 anyway, so numerics are unchanged), which halves HBM traffic
(112 MB -> 60 MB per core), eliminates every on-chip f32->bf16 cast, and
removes the early-phase DMA crunch that starved the PE in v1.

Strategy (per core), all matmuls bf16 (fp32 accumulate in PSUM):
  - GEMM1: stationary = w_gate_up column slices [128h x 128f] (bf16 direct
    load), moving = xT (bf16, loaded in small pieces so the first chain
    starts ~3 us in). GLU = silu(gate) [ACT] * up [DVE] written straight
    into a bf16 SBUF-resident act tile (I, C) - no DRAM round-trip.
  - GEMM2: stationary = act tiles [128i x 128c], moving = w_down h-slabs
    (bf16 direct load, one slab prefetched ahead) -> psum accumulates the
    full 32-tile I chain -> copy -> out (C,H). Slab 0 is prefetched during
    phase 1 (zero phase-boundary bubble).
  - Single PSUM pool for both phases (no pool-release barrier), and all
    matmuls one dtype (avoids the fp32r<->bf16 FWL interleaving hazard
    that crashes the PE).
"""
import numpy as np

E, C, H, I = 8, 1024, 2048, 4096
P = 128
HT, IT, CT = H // P, I // P, C // P  # 16, 32, 8

_CACHE = {}


def _build():
    import concourse.bacc as bacc
    import concourse.mybir as mybir
    import concourse.tile as tile

    f32 = mybir.dt.float32
    bf16 = mybir.dt.bfloat16
    AF = mybir.ActivationFunctionType

    nc = bacc.Bacc("TRN2", target_bir_lowering=False, debug=False)
    xT = nc.declare_dram_parameter("xT", [H, C], bf16, isOutput=False).ap()
    wgu = nc.declare_dram_parameter("wgu", [H, 2 * I], bf16, isOutput=False).ap()
    wdn = nc.declare_dram_parameter("wdn", [I, H], bf16, isOutput=False).ap()
    out = nc.declare_dram_parameter("out", [C, H], f32, isOutput=True).ap()

    xT_v = xT.rearrange("(ht p) c -> p ht c", p=P)    # [128, 16, 1024]
    wgu_v = wgu.rearrange("(ht p) f -> p ht f", p=P)  # [128, 16, 8192]
    wdn_v = wdn.rearrange("(it p) h -> p it h", p=P)  # [128, 32, 2048]
    out_v = out.rearrange("(ct p) h -> p ct h", p=P)  # [128, 8, 2048]

    NHC = 4               # w_down h-slabs
    HW_ = H // NHC        # 512 cols per slab

    with tile.TileContext(nc) as tc:
        with (
            tc.tile_pool(name="acts_pool", bufs=1) as actsp,
            tc.tile_pool(name="pp", bufs=1, space="PSUM") as pp,
            tc.tile_pool(name="sbs", bufs=3) as sbs,
        ):
            # acts[p, it, c] = act row (it*128+p), col c  (bf16, 8 MB)
            acts = actsp.tile([P, IT, C], bf16)

            # slab 0 of w_down is prefetched during phase 1 from this small
            # pool so the phase boundary has zero weight-load bubble
            w2pre_cm = tc.tile_pool(name="w2pre", bufs=1)
            w2pre = w2pre_cm.__enter__()
            wsl0 = w2pre.tile([P, IT, HW_], bf16, name="wsl0")

            # ---- PE warm-up: dummy matmuls on zeroed tiles so the HAM
            # clock-gate reaches 8/8 (2.4 GHz) before the first real chain;
            # they run during the fixed ~6.5us preamble + first-DMA window
            # in which the PE would otherwise idle (and then start cold).
            warm_cm = tc.tile_pool(name="warm", bufs=1)
            warm = warm_cm.__enter__()
            wd = warm.tile([P, P], bf16, name="wd")
            xd = warm.tile([P, 512], bf16, name="xd")
            nc.vector.memset(wd, 0)
            nc.vector.memset(xd, 0)

            def pad(n):
                # dummy matmuls (own PSUM tag; no data deps) — the PE FIFO
                # runs them during DMA waits, keeping the HAM clock warm
                for _ in range(n):
                    pw = pp.tile([P, 512], f32, tag="ps", bufs=4)
                    nc.tensor.matmul(pw, wd, xd, start=True, stop=True)

            pad(8)

            # ---- Phase 1: gate_up GEMM + GLU -> acts ---------------------
            with (
                tc.tile_pool(name="xt_pool", bufs=1) as xtp,
                tc.tile_pool(name="w1", bufs=3) as w1,
            ):
                xt = xtp.tile([P, HT, C], bf16)  # xt[p, ht, c] = x[c, ht*128+p]

                def load_w1(i, which):
                    w = w1.tile([P, HT, P], bf16, tag=which,
                                name=f"{which}{i}")
                    off = 0 if which == "wg" else I
                    nc.sync.dma_start(
                        w, wgu_v[:, :, off + i * P:off + (i + 1) * P])
                    return w

                # Startup: the i=0 chains run in c-quarters (N=256) so the
                # first real matmul needs only ~192KB of data.  The i=0
                # weights live in four SEPARATE tiles (not slices of one
                # tile) so each LDWEIGHTS depends only on its own small DMA
                # - a consumer of one piece of a multi-DMA tile otherwise
                # waits for every writer of that tile.  Pieces are ordered
                # by consumption; first-completion time scales with total
                # bytes in flight, so the early set is kept small.
                wg0a = w1.tile([P, 4, P], bf16, name="wg0a")
                wg0b = w1.tile([P, HT - 4, P], bf16, name="wg0b")
                wu0a = w1.tile([P, 8, P], bf16, name="wu0a")
                wu0b = w1.tile([P, 8, P], bf16, name="wu0b")
                nc.sync.dma_start(xt[:, 0:2, 0:256], xT_v[:, 0:2, 0:256])
                nc.sync.dma_start(wg0a, wgu_v[:, 0:4, 0:P])
                nc.sync.dma_start(xt[:, 2:8, 0:256], xT_v[:, 2:8, 0:256])
                nc.sync.dma_start(wg0b, wgu_v[:, 4:16, 0:P])
                nc.sync.dma_start(xt[:, 8:16, 0:256], xT_v[:, 8:16, 0:256])
                nc.sync.dma_start(wu0a, wgu_v[:, 0:8, I:I + P])
                nc.sync.dma_start(wu0b, wgu_v[:, 8:16, I:I + P])
                for q in range(1, 4):
                    cs = slice(q * 256, (q + 1) * 256)
                    nc.sync.dma_start(xt[:, :, cs], xT_v[:, :, cs])

                def wg0_at(ht):
                    return wg0a[:, ht, :] if ht < 4 else wg0b[:, ht - 4, :]

                def wu0_at(ht):
                    return wu0a[:, ht, :] if ht < 8 else wu0b[:, ht - 8, :]

                wg_n = load_w1(1, "wg")
                wu_n = load_w1(1, "wu")
                for i in range(IT):
                    if i > 0:
                        wg, wu = wg_n, wu_n
                        if i + 1 < IT:
                            wg_n = load_w1(i + 1, "wg")
                            wu_n = load_w1(i + 1, "wu")
                    if 8 <= i < 12:
                        # slab-0 quarter loads, spread mid-phase-1
                        q = i - 8
                        qs = slice(q * (IT // 4), (q + 1) * (IT // 4))
                        nc.sync.dma_start(wsl0[:, qs, :], wdn_v[:, qs, 0:HW_])
                    ncc, cw = (4, 256) if i == 0 else (2, 512)
                    for cc in range(ncc):
                        cs = slice(cc * cw, (cc + 1) * cw)
                        if i == 0 and cc == 2:
                            pad(3)  # x quarter-2 arrival wait
                        pg = pp.tile([P, cw], f32, tag="pg", bufs=2)
                        pu = pp.tile([P, cw], f32, tag="pu", bufs=2)
                        for ht in range(HT):
                            wga = wg0_at(ht) if i == 0 else wg[:, ht, :]
                            nc.tensor.matmul(
                                pg, wga, xt[:, ht, cs],
                                start=(ht == 0), stop=(ht == HT - 1))
                            if i == 0 and cc == 0 and ht == 1:
                                pad(3)  # xt[2:8] arrival wait
                            if i == 0 and cc == 0 and ht == 3:
                                pad(2)  # wg0b arrival wait
                        if i == 0 and cc == 0:
                            pad(2)  # wu0 arrival wait
                        for ht in range(HT):
                            wua = wu0_at(ht) if i == 0 else wu[:, ht, :]
                            nc.tensor.matmul(
                                pu, wua, xt[:, ht, cs],
                                start=(ht == 0), stop=(ht == HT - 1))
                        sil = sbs.tile([P, cw], f32, tag="sil")
                        nc.scalar.activation(sil, pg, AF.Silu)
                        nc.vector.tensor_mul(acts[:, i, cs], sil, pu)

            # ---- Phase 2: down GEMM (bf16), full-I psum chains -----------
            with tc.tile_pool(name="w2", bufs=2) as w2:
                def load_w2(hc):
                    hs = slice(hc * HW_, (hc + 1) * HW_)
                    wsl = w2.tile([P, IT, HW_], bf16, tag="wsl",
                                  name=f"wsl{hc}")
                    for q in range(2):
                        qs = slice(q * (IT // 2), (q + 1) * (IT // 2))
                        nc.sync.dma_start(wsl[:, qs, :], wdn_v[:, qs, hs])
                    return wsl

                wsl_n = load_w2(1)
                for hc in range(NHC):
                    hs = slice(hc * HW_, (hc + 1) * HW_)
                    if hc == 0:
                        wsl = wsl0
                    else:
                        wsl = wsl_n
                        if hc + 1 < NHC:
                            wsl_n = load_w2(hc + 1)
                    for ct in range(CT):
                        ps = pp.tile([P, HW_], f32, tag="ps", bufs=4)
                        for i in range(IT):
                            nc.tensor.matmul(
                                ps,
                                acts[:, i, ct * P:(ct + 1) * P],
                                wsl[:, i, :],
                                start=(i == 0), stop=(i == IT - 1))
                        osb = sbs.tile([P, HW_], f32, tag="osb")
                        nc.vector.tensor_copy(osb, ps)
                        nc.sync.dma_start(out_v[:, ct, hs], osb)
            warm_cm.__exit__(None, None, None)
            w2pre_cm.__exit__(None, None, None)

    nc.compile()
    return nc


def _get_nc():
    if "nc" not in _CACHE:
        _CACHE["nc"] = _build()
    return _CACHE["nc"]


def _run(hidden_states, w_gate_up, w_down, trace=False):
    import ml_dtypes
    from concourse.bass_utils import run_bass_kernel_spmd

    nc = _get_nc()
    bf = ml_dtypes.bfloat16
    hs = np.asarray(hidden_states, dtype=np.float32)
    wg = np.asarray(w_gate_up, dtype=np.float32).astype(bf)
    wd = np.asarray(w_down, dtype=np.float32).astype(bf)
    in_maps = [
        {
            "xT": np.ascontiguousarray(hs[e].T.astype(bf)),
            "wgu": np.ascontiguousarray(wg[e]),
            "wdn": np.ascontiguousarray(wd[e]),
        }
        for e in range(E)
    ]
    res = run_bass_kernel_spmd(nc, in_maps, list(range(E)), trace=trace)
    output = np.stack([res.results[e]["out"] for e in range(E)], axis=0)
    return output, res


def kernel(hidden_states, w_gate_up, w_down):
    output, _ = _run(hidden_states, w_gate_up, w_down, trace=False)
    return output


# revision 18
# speedup vs baseline: 1.0084x; 1.0084x over previous
"""Expert-parallel MoE GLU MLP kernel for Trainium2.

8 experts -> 8 NeuronCores, one expert per core (no collectives needed).
Per core:  x (C,H) @ w_gate_up (H,2I) -> GLU -> (C,I) @ w_down (I,H) -> (C,H)

v2: all inputs are pre-cast to bf16 on the host (the kernel computed in
bf16 anyway, so numerics are unchanged), which halves HBM traffic
(112 MB -> 60 MB per core), eliminates every on-chip f32->bf16 cast, and
removes the early-phase DMA crunch that starved the PE in v1.

Strategy (per core), all matmuls bf16 (fp32 accumulate in PSUM):
  - GEMM1: stationary = w_gate_up column slices [128h x 128f] (bf16 direct
    load), moving = xT (bf16, loaded in small pieces so the first chain
    starts ~3 us in). GLU = silu(gate) [ACT] * up [DVE] written straight
    into a bf16 SBUF-resident act tile (I, C) - no DRAM round-trip.
  - GEMM2: stationary = act tiles [128i x 128c], moving = w_down h-slabs
    (bf16 direct load, one slab prefetched ahead) -> psum accumulates the
    full 32-tile I chain -> copy -> out (C,H). Slab 0 is prefetched during
    phase 1 (zero phase-boundary bubble).
  - Single PSUM pool for both phases (no pool-release barrier), and all
    matmuls one dtype (avoids the fp32r<->bf16 FWL interleaving hazard
    that crashes the PE).
"""
import numpy as np

E, C, H, I = 8, 1024, 2048, 4096
P = 128
HT, IT, CT = H // P, I // P, C // P  # 16, 32, 8

_CACHE = {}


def _build():
    import concourse.bacc as bacc
    import concourse.mybir as mybir
    import concourse.tile as tile

    f32 = mybir.dt.float32
    bf16 = mybir.dt.bfloat16
    AF = mybir.ActivationFunctionType

    nc = bacc.Bacc("TRN2", target_bir_lowering=False, debug=False)
    xT = nc.declare_dram_parameter("xT", [H, C], bf16, isOutput=False).ap()
    wgu = nc.declare_dram_parameter("wgu", [H, 2 * I], bf16, isOutput=False).ap()
    wdn = nc.declare_dram_parameter("wdn", [I, H], bf16, isOutput=False).ap()
    out = nc.declare_dram_parameter("out", [C, H], f32, isOutput=True).ap()

    xT_v = xT.rearrange("(ht p) c -> p ht c", p=P)    # [128, 16, 1024]
    wgu_v = wgu.rearrange("(ht p) f -> p ht f", p=P)  # [128, 16, 8192]
    wdn_v = wdn.rearrange("(it p) h -> p it h", p=P)  # [128, 32, 2048]
    out_v = out.rearrange("(ct p) h -> p ct h", p=P)  # [128, 8, 2048]

    NHC = 4               # w_down h-slabs
    HW_ = H // NHC        # 512 cols per slab

    with tile.TileContext(nc) as tc:
        with (
            tc.tile_pool(name="acts_pool", bufs=1) as actsp,
            tc.tile_pool(name="pp", bufs=1, space="PSUM") as pp,
            tc.tile_pool(name="sbs", bufs=3) as sbs,
        ):
            # acts[p, it, c] = act row (it*128+p), col c  (bf16, 8 MB)
            acts = actsp.tile([P, IT, C], bf16)

            # slab 0 of w_down is prefetched during phase 1 from this small
            # pool so the phase boundary has zero weight-load bubble
            w2pre_cm = tc.tile_pool(name="w2pre", bufs=1)
            w2pre = w2pre_cm.__enter__()
            wsl0 = w2pre.tile([P, IT, HW_], bf16, name="wsl0")

            # ---- PE warm-up: dummy matmuls on zeroed tiles so the HAM
            # clock-gate reaches 8/8 (2.4 GHz) before the first real chain;
            # they run during the fixed ~6.5us preamble + first-DMA window
            # in which the PE would otherwise idle (and then start cold).
            warm_cm = tc.tile_pool(name="warm", bufs=1)
            warm = warm_cm.__enter__()
            wd = warm.tile([P, P], bf16, name="wd")
            xd = warm.tile([P, 512], bf16, name="xd")
            nc.vector.memset(wd, 0)
            nc.vector.memset(xd, 0)
            for _ in range(10):
                pw = pp.tile([P, 512], f32, tag="pg", bufs=2)
                nc.tensor.matmul(pw, wd, xd, start=True, stop=True)

            # ---- Phase 1: gate_up GEMM + GLU -> acts ---------------------
            with (
                tc.tile_pool(name="xt_pool", bufs=1) as xtp,
                tc.tile_pool(name="w1", bufs=3) as w1,
            ):
                xt = xtp.tile([P, HT, C], bf16)  # xt[p, ht, c] = x[c, ht*128+p]

                def load_w1(i, which):
                    w = w1.tile([P, HT, P], bf16, tag=which,
                                name=f"{which}{i}")
                    off = 0 if which == "wg" else I
                    nc.sync.dma_start(
                        w, wgu_v[:, :, off + i * P:off + (i + 1) * P])
                    return w

                # Startup: first-chain weights and the cc=0 half of x are
                # interleaved on the Sync HWDGE queue, pieces ordered so
                # data arrives just ahead of the consuming LDW/MATMUL.
                wg_n = w1.tile([P, HT, P], bf16, tag="wg", name="wg0")
                wu_n = w1.tile([P, HT, P], bf16, tag="wu", name="wu0")
                nc.sync.dma_start(wg_n[:, 0:4, :], wgu_v[:, 0:4, 0:P])
                nc.sync.dma_start(xt[:, 0:2, 0:512], xT_v[:, 0:2, 0:512])
                nc.sync.dma_start(wg_n[:, 4:16, :], wgu_v[:, 4:16, 0:P])
                nc.sync.dma_start(xt[:, 2:8, 0:512], xT_v[:, 2:8, 0:512])
                nc.sync.dma_start(wu_n[:, 0:8, :], wgu_v[:, 0:8, I:I + P])
                nc.sync.dma_start(xt[:, 8:16, 0:512], xT_v[:, 8:16, 0:512])
                nc.sync.dma_start(wu_n[:, 8:16, :], wgu_v[:, 8:16, I:I + P])
                nc.sync.dma_start(xt[:, 0:8, 512:1024],
                                    xT_v[:, 0:8, 512:1024])
                nc.sync.dma_start(xt[:, 8:16, 512:1024],
                                    xT_v[:, 8:16, 512:1024])

                for i in range(IT):
                    wg, wu = wg_n, wu_n
                    if i + 1 < IT:
                        wg_n = load_w1(i + 1, "wg")
                        wu_n = load_w1(i + 1, "wu")
                    if 8 <= i < 12:
                        # slab-0 quarter loads, spread mid-phase-1
                        q = i - 8
                        qs = slice(q * (IT // 4), (q + 1) * (IT // 4))
                        nc.sync.dma_start(wsl0[:, qs, :], wdn_v[:, qs, 0:HW_])
                    for cc in range(2):
                        cs = slice(cc * 512, (cc + 1) * 512)
                        pg = pp.tile([P, 512], f32, tag="pg", bufs=2)
                        pu = pp.tile([P, 512], f32, tag="pu", bufs=2)
                        for ht in range(HT):
                            nc.tensor.matmul(
                                pg, wg[:, ht, :], xt[:, ht, cs],
                                start=(ht == 0), stop=(ht == HT - 1))
                        for ht in range(HT):
                            nc.tensor.matmul(
                                pu, wu[:, ht, :], xt[:, ht, cs],
                                start=(ht == 0), stop=(ht == HT - 1))
                        sil = sbs.tile([P, 512], f32, tag="sil")
                        nc.scalar.activation(sil, pg, AF.Silu)
                        nc.vector.tensor_mul(acts[:, i, cs], sil, pu)

            # ---- Phase 2: down GEMM (bf16), full-I psum chains -----------
            with tc.tile_pool(name="w2", bufs=2) as w2:
                def load_w2(hc):
                    hs = slice(hc * HW_, (hc + 1) * HW_)
                    wsl = w2.tile([P, IT, HW_], bf16, tag="wsl",
                                  name=f"wsl{hc}")
                    for q in range(2):
                        qs = slice(q * (IT // 2), (q + 1) * (IT // 2))
                        nc.sync.dma_start(wsl[:, qs, :], wdn_v[:, qs, hs])
                    return wsl

                wsl_n = load_w2(1)
                for hc in range(NHC):
                    hs = slice(hc * HW_, (hc + 1) * HW_)
                    if hc == 0:
                        wsl = wsl0
                    else:
                        wsl = wsl_n
                        if hc + 1 < NHC:
                            wsl_n = load_w2(hc + 1)
                    for ct in range(CT):
                        ps = pp.tile([P, HW_], f32, tag="ps", bufs=4)
                        for i in range(IT):
                            nc.tensor.matmul(
                                ps,
                                acts[:, i, ct * P:(ct + 1) * P],
                                wsl[:, i, :],
                                start=(i == 0), stop=(i == IT - 1))
                        osb = sbs.tile([P, HW_], f32, tag="osb")
                        nc.vector.tensor_copy(osb, ps)
                        nc.sync.dma_start(out_v[:, ct, hs], osb)
            warm_cm.__exit__(None, None, None)
            w2pre_cm.__exit__(None, None, None)

    nc.compile()
    return nc


def _get_nc():
    if "nc" not in _CACHE:
        _CACHE["nc"] = _build()
    return _CACHE["nc"]


def _run(hidden_states, w_gate_up, w_down, trace=False):
    import ml_dtypes
    from concourse.bass_utils import run_bass_kernel_spmd

    nc = _get_nc()
    bf = ml_dtypes.bfloat16
    hs = np.asarray(hidden_states, dtype=np.float32)
    wg = np.asarray(w_gate_up, dtype=np.float32).astype(bf)
    wd = np.asarray(w_down, dtype=np.float32).astype(bf)
    in_maps = [
        {
            "xT": np.ascontiguousarray(hs[e].T.astype(bf)),
            "wgu": np.ascontiguousarray(wg[e]),
            "wdn": np.ascontiguousarray(wd[e]),
        }
        for e in range(E)
    ]
    res = run_bass_kernel_spmd(nc, in_maps, list(range(E)), trace=trace)
    output = np.stack([res.results[e]["out"] for e in range(E)], axis=0)
    return output, res


def kernel(hidden_states, w_gate_up, w_down):
    output, _ = _run(hidden_states, w_gate_up, w_down, trace=False)
    return output
